# revision 10
# baseline (speedup 1.0000x reference)
"""Multi-head attention (N=4, T=2048, D=512, H=8, dh=64) on 8 TRN2 NeuronCores.

Sharding: batch N (4) x head-group (2 groups of 4 heads) -> 8 cores.

v3c: scores and AV run in the PE's 64x128 2-tile mode (T0 = SBUF
partitions 0-63, T8 = 64-127); adjacent T0/T8 matmuls execute
concurrently (~2x). Projections stay full-array K=128 (infrequent, so
their mode switches are cheap). PSUM discipline: a bank is never open
on two row tiles at once -- AV accumulates each head's [65, 512] block
in ONE bank via two time-separated half-contraction sweeps (T0 sweep,
explicit LDWEIGHTS separator, T8 sweep).

exp is split per head: k-tiles 0..7 on VectorE (Schraudolph bit trick
bf16 = bitcast_i16(round(A*s + B)), one fused tensor_scalar op),
k-tiles 8..15 on ScalarE (native Exp, 1024-wide chunks). Row 64 of each
output block is the softmax denominator; the HOST divides + transposes.
"""

import math

import ml_dtypes
import numpy as np

import concourse.bass as bass
import concourse.mybir as mybir
import concourse.tile as tile
from concourse import bacc
from concourse.bass_utils import run_bass_kernel_spmd

F32 = mybir.dt.float32
BF16 = mybir.dt.bfloat16
I16 = mybir.dt.int16
EXP = mybir.ActivationFunctionType.Exp
MULT = mybir.AluOpType.mult
ADD = mybir.AluOpType.add

N, T, D = 4, 2048, 512
HPC, DH = 4, 64
GC = HPC * DH
SCALE = 1.0 / math.sqrt(D)
QB = 512
NQB = T // QB            # 4
NKT = T // 128           # 16
KS = D // 128            # 4
OROW = DH + 1            # 65

DVE_KT = 8               # k-tiles 0..7 exp'd by VectorE per head
SCH_C = 4.0
SCH_A = (128.0 / math.log(2.0)) * SCALE
SCH_B = 127.0 * 128.0 - SCH_C


def build():
    nc = bacc.Bacc("TRN2", target_bir_lowering=False, debug=False, num_devices=8)
    qT_in = nc.declare_dram_parameter("qT", [D, T], BF16, isOutput=False)
    kT_in = nc.declare_dram_parameter("kT", [D, T], BF16, isOutput=False)
    wq_in = nc.declare_dram_parameter("wq", [128, KS * GC], BF16, isOutput=False)
    wk_in = nc.declare_dram_parameter("wk", [128, KS * GC], BF16, isOutput=False)
    wv_in = nc.declare_dram_parameter("wv", [128, KS * GC], BF16, isOutput=False)
    oT_out = nc.declare_dram_parameter("oT65", [HPC * OROW, T], F32, isOutput=True)

    with tile.TileContext(nc) as tc:
        with (
            tc.tile_pool(name="stage", bufs=8) as stage,
            tc.tile_pool(name="const", bufs=1) as const,
            tc.tile_pool(name="act", bufs=1) as actp,
            tc.tile_pool(name="ptl", bufs=2) as ptlp,
            tc.tile_pool(name="pth", bufs=2) as pthp,
            tc.tile_pool(name="ost", bufs=4) as ostp,
            tc.tile_pool(name="ring", bufs=2, space="PSUM") as ring,  # 4 banks
            tc.tile_pool(name="psP", bufs=2, space="PSUM") as psP,    # 2 banks
            tc.tile_pool(name="psO", bufs=2, space="PSUM") as psO,    # 2 banks
        ):
            # ---- input staging ----
            kin = [stage.tile([128, T], BF16, tag="qkin", name=f"kin{s}")
                   for s in range(KS)]
            qin = [stage.tile([128, T], BF16, tag="qkin", name=f"qin{s}")
                   for s in range(KS)]
            wv = const.tile([128, KS, GC], BF16, tag="wv")
            wk = const.tile([128, KS, GC], BF16, tag="wk")
            wq = const.tile([128, KS, GC], BF16, tag="wq")

            for s in range(KS):
                nc.sync.dma_start(
                    kin[s][:, 0:QB], kT_in[s * 128 : (s + 1) * 128, 0:QB])
            nc.sync.dma_start(wv[:], wv_in.rearrange("p (s c) -> p s c", s=KS))
            nc.sync.dma_start(wk[:], wk_in.rearrange("p (s c) -> p s c", s=KS))
            for tb in range(1, NQB):
                for s in range(KS):
                    nc.sync.dma_start(
                        kin[s][:, tb * QB : (tb + 1) * QB],
                        kT_in[s * 128 : (s + 1) * 128, tb * QB : (tb + 1) * QB])
            nc.sync.dma_start(wq[:], wq_in.rearrange("p (s c) -> p s c", s=KS))
            for tb in range(NQB):
                for s in range(KS):
                    nc.sync.dma_start(
                        qin[s][:, tb * QB : (tb + 1) * QB],
                        qT_in[s * 128 : (s + 1) * 128, tb * QB : (tb + 1) * QB])

            kT_att = [actp.tile([128, T], BF16, tag=f"ka{d}", name=f"ka{d}")
                      for d in range(2)]
            qT_att = [actp.tile([128, T], BF16, tag=f"qa{d}", name=f"qa{d}")
                      for d in range(2)]

            vp = const.tile([128, NKT, HPC, OROW], BF16, tag="vp")
            zw = const.tile([128, OROW], BF16, tag="zw")
            nc.gpsimd.memset(zw[:], 0.0)
            ones_f32 = const.tile([128, NKT * HPC], F32, tag="ones")
            nc.gpsimd.memset(ones_f32[:], 1.0)
            nc.vector.tensor_copy(
                vp[:, :, :, DH : DH + 1],
                ones_f32[:].rearrange("p (a b) -> p a b", b=HPC).unsqueeze(3))

            # ---- projections: full-array K=128 (v2 style) ----
            def emit_kqproj(which, tb):
                w, src, dst = (
                    (wk, kin, kT_att) if which == "k" else (wq, qin, qT_att))
                cols = slice(tb * QB, (tb + 1) * QB)
                for dt2 in range(2):
                    ps = psP.tile([128, QB], F32, tag="P",
                                  name=f"{which}p{tb}_{dt2}")
                    for s in range(KS):
                        nc.tensor.matmul(
                            ps[:], w[:, s, dt2 * 128 : (dt2 + 1) * 128],
                            src[s][:, cols], start=(s == 0), stop=(s == KS - 1))
                    nc.vector.tensor_copy(dst[dt2][:, cols], ps[:])

            def emit_vproj(tt):
                ps = psP.tile([128, QB], F32, tag="P", name=f"vp{tt}")
                for s in range(KS):
                    nc.tensor.matmul(
                        ps[:, 0:GC], kin[s][:, tt * 128 : (tt + 1) * 128],
                        wv[:, s, :], start=(s == 0), stop=(s == KS - 1))
                nc.vector.tensor_copy(
                    vp[:, tt, :, 0:DH],
                    ps[:, 0:GC].rearrange("p (h d) -> p h d", d=DH))

            for tt in range(4):
                emit_vproj(tt)
            for tb in range(NQB):
                emit_kqproj("k", tb)
            emit_kqproj("q", 0)

            # ---- attention ----
            def emit_scores(t2, qb, pt_lo, pt_hi):
                q_lo = qT_att[t2][0:DH, qb * QB : (qb + 1) * QB]
                q_hi = qT_att[t2][DH:128, qb * QB : (qb + 1) * QB]
                for c in range(NKT // 2):
                    kt0 = 2 * c
                    sl = ring.tile([128, 2 * QB], F32, tag="R", name=f"sc_lo{c}")
                    sh = ring.tile([128, 2 * QB], F32, tag="R", name=f"sc_hi{c}")
                    for l in range(2):
                        kt = kt0 + l
                        nc.tensor.matmul(
                            sl[:, l * QB : (l + 1) * QB],
                            kT_att[t2][0:DH, kt * 128 : (kt + 1) * 128],
                            q_lo, start=True, stop=True)
                        nc.tensor.matmul(
                            sh[:, l * QB : (l + 1) * QB],
                            kT_att[t2][DH:128, kt * 128 : (kt + 1) * 128],
                            q_hi, start=True, stop=True)
                    osl = slice(kt0 * QB, (kt0 + 2) * QB)
                    if kt0 < DVE_KT:
                        nc.vector.tensor_scalar(
                            pt_lo[:, osl].bitcast(I16), sl[:],
                            SCH_A, SCH_B, MULT, ADD)
                        nc.vector.tensor_scalar(
                            pt_hi[:, osl].bitcast(I16), sh[:],
                            SCH_A, SCH_B, MULT, ADD)
                    else:
                        nc.scalar.activation(pt_lo[:, osl], sl[:], EXP,
                                             scale=SCALE)
                        nc.scalar.activation(pt_hi[:, osl], sh[:], EXP,
                                             scale=SCALE)

            def emit_av(t2, qb, pt_lo, pt_hi):
                hp_lo, hp_hi = 2 * t2, 2 * t2 + 1
                po_lo = psO.tile([128, QB], F32, tag="O", name="po_lo")
                po_hi = psO.tile([128, QB], F32, tag="O", name="po_hi")
                for kt in range(NKT):
                    ksl = slice(kt * QB, (kt + 1) * QB)
                    nc.tensor.matmul(
                        po_lo[0:OROW], vp[:, kt, hp_lo, :],
                        pt_lo[:, ksl], start=(kt == 0), stop=(kt == NKT - 1))
                    nc.tensor.matmul(
                        po_hi[0:OROW], vp[:, kt, hp_hi, :],
                        pt_hi[:, ksl], start=(kt == 0), stop=(kt == NKT - 1))
                return po_lo, po_hi

            def emit_out(t2, qb, po, which):
                hp = 2 * t2 + (0 if which == "lo" else 1)
                st = ostp.tile([128, QB], F32, tag="ost", name=f"o{which}")
                if which == "lo":
                    nc.scalar.copy(st[0:OROW, :], po[0:OROW, :])
                else:
                    nc.vector.tensor_copy(st[0:OROW, :], po[0:OROW, :])
                nc.gpsimd.dma_start(
                    oT_out[hp * OROW : (hp + 1) * OROW,
                           qb * QB : (qb + 1) * QB],
                    st[0:OROW, :])

            sps = [(qb, t2) for qb in range(NQB) for t2 in range(2)]
            for i, (qb, t2) in enumerate(sps):
                pt_lo = ptlp.tile([128, NKT * QB], BF16, tag="ptl", name="ptl")
                pt_hi = pthp.tile([128, NKT * QB], BF16, tag="pth", name="pth")
                emit_scores(t2, qb, pt_lo, pt_hi)
                if i == 0:
                    for tt in range(4, NKT):
                        emit_vproj(tt)
                if t2 == 1 and qb + 1 < NQB:
                    emit_kqproj("q", qb + 1)
                po_lo, po_hi = emit_av(t2, qb, pt_lo, pt_hi)
                emit_out(t2, qb, po_lo, "lo")
                emit_out(t2, qb, po_hi, "hi")

    nc.compile()
    return nc


_NC = None


def _get_nc():
    global _NC
    if _NC is None:
        _NC = build()
    return _NC


def _prep_w(W, cols):
    w = W[:, cols].astype(ml_dtypes.bfloat16)           # [512, 256]
    w = w.reshape(KS, 128, GC).transpose(1, 0, 2)       # [128, KS, GC]
    return np.ascontiguousarray(w.reshape(128, KS * GC))


def run(query, key, W_query, W_key, W_value, trace=False):
    nc = _get_nc()
    query = np.asarray(query, dtype=np.float32)
    key = np.asarray(key, dtype=np.float32)
    W_query = np.asarray(W_query, dtype=np.float32)
    W_key = np.asarray(W_key, dtype=np.float32)
    W_value = np.asarray(W_value, dtype=np.float32)

    in_maps = []
    for c in range(8):
        n, g = c // 2, c % 2
        cols = slice(g * GC, (g + 1) * GC)
        in_maps.append(
            {
                "qT": np.ascontiguousarray(query[n].T.astype(ml_dtypes.bfloat16)),
                "kT": np.ascontiguousarray(key[n].T.astype(ml_dtypes.bfloat16)),
                "wq": _prep_w(W_query, cols),
                "wk": _prep_w(W_key, cols),
                "wv": _prep_w(W_value, cols),
            }
        )
    res = run_bass_kernel_spmd(nc, in_maps, core_ids=list(range(8)), trace=trace)
    out = np.empty((N, T, D), dtype=np.float32)
    for c in range(8):
        n, g = c // 2, c % 2
        r = res.results[c]["oT65"]  # [4*65, 2048]
        for hp in range(HPC):
            blk = r[hp * OROW : (hp + 1) * OROW]
            out[n, :, g * GC + hp * DH : g * GC + (hp + 1) * DH] = (
                blk[0:DH] / blk[DH : DH + 1]
            ).T
    return out, res


def kernel(query, key, W_query, W_key, W_value):
    out, _ = run(query, key, W_query, W_key, W_value, trace=False)
    return out
